# revision 1
# baseline (speedup 1.0000x reference)
"""Block-diagonal linear layer (16 blocks of 256x256) on 8 TRN2 NeuronCores.

Strategy: data-parallel over the batch dim (16384 rows -> 2048 rows/core).
The TensorEngine contracts over the partition dim, so x is pre-transposed on
the host to feature-major layout ([4096, rows]); the kernel computes
yT[o, n] = sum_i W[k, o, i] * xT[k*256+i, n] + b[k, o] per block and the host
transposes the gathered output back. Block weights are replicated, packed
host-side into a single [128, 8192] SBUF-ready image so they load in one DMA.
"""

import sys

import numpy as np

try:
    import concourse  # noqa: F401
except ImportError:
    sys.path.insert(0, "/opt/trn_rl_repo")

NUM_BLOCKS = 16
IN_FEATURES = 4096
OUT_FEATURES = 4096
BLOCK_IN = 256
BLOCK_OUT = 256
BATCH = 16384
NCORES = 8
ROWS = BATCH // NCORES  # rows per core

# test.py toggles these for profiling.
TRACE = False
LAST_EXEC_NS = None

_BUILT = {}


def _build(rows: int):
    """Build + compile the single-core Bass program (identical SPMD on 8 cores)."""
    import concourse.mybir as mybir
    import concourse.tile as tile
    from concourse import bacc

    nc = bacc.Bacc("TRN2", target_bir_lowering=False, debug=False)
    f32 = mybir.dt.float32
    f32r = mybir.dt.float32r

    xT = nc.dram_tensor("xT", [IN_FEATURES, rows], f32r, kind="ExternalInput").ap()
    Wh = nc.dram_tensor("Wh", [128, 32 * 256], f32r, kind="ExternalInput").ap()
    bh = nc.dram_tensor("bh", [128, 32], f32, kind="ExternalInput").ap()
    yT = nc.dram_tensor("yT", [OUT_FEATURES, rows], f32, kind="ExternalOutput").ap()

    NFREE = 512  # one fp32 PSUM bank / max fp32 moving dim
    n_chunks = rows // NFREE

    with tile.TileContext(nc) as tc:
        with (
            tc.tile_pool(name="wp", bufs=1) as wpool,
            tc.tile_pool(name="xp", bufs=6) as xpool,
            tc.tile_pool(name="yp", bufs=4) as ypool,
            tc.tile_pool(name="pp", bufs=8, space="PSUM") as ppool,
        ):
            w_all = wpool.tile([128, 32 * 256], f32r)
            nc.sync.dma_start(out=w_all[:], in_=Wh[:])
            bias_sb = wpool.tile([128, 32], f32)
            nc.sync.dma_start(out=bias_sb[:], in_=bh[:])

            for k in range(NUM_BLOCKS):
                xt = []
                for i2 in range(2):
                    t = xpool.tile([128, rows], f32r, tag="xt")
                    r0 = (k * 2 + i2) * 128
                    nc.sync.dma_start(out=t[:], in_=xT[r0 : r0 + 128, :])
                    xt.append(t)
                for o2 in range(2):
                    c = k * 2 + o2
                    y_sb = ypool.tile([128, rows], f32, tag="yt")
                    for n4 in range(n_chunks):
                        ps = ppool.tile([128, NFREE], f32)
                        for i2 in range(2):
                            w0 = (k * 2 + i2) * 256 + o2 * 128
                            nc.tensor.matmul(
                                ps[:],
                                lhsT=w_all[:, w0 : w0 + 128],
                                rhs=xt[i2][:, n4 * NFREE : (n4 + 1) * NFREE],
                                start=(i2 == 0),
                                stop=(i2 == 1),
                            )
                        nc.scalar.activation(
                            y_sb[:, n4 * NFREE : (n4 + 1) * NFREE],
                            ps[:],
                            mybir.ActivationFunctionType.Identity,
                            bias=bias_sb[:, c : c + 1],
                        )
                    nc.sync.dma_start(out=yT[c * 128 : (c + 1) * 128, :], in_=y_sb[:])

    nc.compile()
    return nc


def _get_nc(rows: int):
    if rows not in _BUILT:
        _BUILT[rows] = _build(rows)
    return _BUILT[rows]


def kernel(x: np.ndarray, W: np.ndarray, b: np.ndarray) -> np.ndarray:
    global LAST_EXEC_NS
    from concourse.bass_utils import run_bass_kernel_spmd

    assert x.shape == (BATCH, IN_FEATURES) and x.dtype == np.float32
    nc = _get_nc(ROWS)

    # Weight image: Wh[p, (k*2+i2)*256 + o] = W[k, o, i2*128+p]
    Wh = np.ascontiguousarray(
        W.transpose(0, 2, 1)  # [k, i, o]
        .reshape(NUM_BLOCKS, 2, 128, BLOCK_OUT)  # [k, i2, p, o]
        .transpose(2, 0, 1, 3)  # [p, k, i2, o]
        .reshape(128, 32 * 256)
    ).astype(np.float32, copy=False)
    # Bias image: bh[p, k*2+o2] = b[k, o2*128+p]
    bh = np.ascontiguousarray(
        b.reshape(NUM_BLOCKS, 2, 128).transpose(2, 0, 1).reshape(128, 32)
    ).astype(np.float32, copy=False)

    xs = x.reshape(NCORES, ROWS, IN_FEATURES)
    in_maps = [
        {
            "xT": np.ascontiguousarray(xs[c].T),
            "Wh": Wh,
            "bh": bh,
        }
        for c in range(NCORES)
    ]

    res = run_bass_kernel_spmd(nc, in_maps, list(range(NCORES)), trace=TRACE)
    LAST_EXEC_NS = res.exec_time_ns

    y = np.empty((BATCH, OUT_FEATURES), dtype=np.float32)
    for c in range(NCORES):
        y[c * ROWS : (c + 1) * ROWS, :] = res.results[c]["yT"].T
    return y


# revision 2
# speedup vs baseline: 1.9548x; 1.9548x over previous
"""Block-diagonal linear layer (16 blocks of 256x256) on 8 TRN2 NeuronCores.

Strategy: data-parallel over the batch dim (16384 rows -> 2048 rows/core).
The TensorEngine contracts over the partition dim, so x is pre-transposed on
the host to feature-major layout ([4096, rows]); the kernel computes
yT[o, n] = sum_i W[k, o, i] * xT[k*256+i, n] + b[k, o] per block and the host
transposes the gathered output back. Block weights are replicated, packed
host-side into a single [128, 8192] SBUF-ready image so they load in one DMA.

The kernel is memory-bound (71MB of f32 DMA traffic per core), so x/W/y are
carried as bf16 on the wire (PSUM accumulation stays f32); measured rel err
vs the f32 reference is ~2e-3.
"""

import sys

import numpy as np

try:
    import concourse  # noqa: F401
except ImportError:
    sys.path.insert(0, "/opt/trn_rl_repo")

NUM_BLOCKS = 16
IN_FEATURES = 4096
OUT_FEATURES = 4096
BLOCK_IN = 256
BLOCK_OUT = 256
BATCH = 16384
NCORES = 8
ROWS = BATCH // NCORES  # rows per core

# "bf16": x/W/y bf16 on the wire, f32 PSUM accumulate (fast, rel err ~2e-3)
# "f32r": everything f32, matmul in float32r mode (rel err ~1e-4)
MODE = "bf16"

# test.py toggles these for profiling.
TRACE = False
LAST_EXEC_NS = None

_BUILT = {}


def _build(rows: int, mode: str):
    """Build + compile the single-core Bass program (identical SPMD on 8 cores)."""
    import concourse.mybir as mybir
    import concourse.tile as tile
    from concourse import bacc

    nc = bacc.Bacc("TRN2", target_bir_lowering=False, debug=False)
    f32 = mybir.dt.float32
    mm_dt = mybir.dt.bfloat16 if mode == "bf16" else mybir.dt.float32r
    out_dt = mybir.dt.bfloat16 if mode == "bf16" else f32

    xT = nc.dram_tensor("xT", [IN_FEATURES, rows], mm_dt, kind="ExternalInput").ap()
    Wh = nc.dram_tensor("Wh", [128, 32 * 256], mm_dt, kind="ExternalInput").ap()
    bh = nc.dram_tensor("bh", [128, 32], f32, kind="ExternalInput").ap()
    yT = nc.dram_tensor("yT", [OUT_FEATURES, rows], out_dt, kind="ExternalOutput").ap()

    NFREE = 512  # one fp32 PSUM bank
    n_chunks = rows // NFREE

    with tile.TileContext(nc) as tc:
        with (
            tc.tile_pool(name="wp", bufs=1) as wpool,
            tc.tile_pool(name="xp", bufs=6) as xpool,
            tc.tile_pool(name="yp", bufs=4) as ypool,
            tc.tile_pool(name="pp", bufs=8, space="PSUM") as ppool,
        ):
            w_all = wpool.tile([128, 32 * 256], mm_dt)
            nc.sync.dma_start(out=w_all[:], in_=Wh[:])
            bias_sb = wpool.tile([128, 32], f32)
            nc.sync.dma_start(out=bias_sb[:], in_=bh[:])

            for k in range(NUM_BLOCKS):
                xt = []
                for i2 in range(2):
                    t = xpool.tile([128, rows], mm_dt, tag="xt")
                    r0 = (k * 2 + i2) * 128
                    nc.sync.dma_start(out=t[:], in_=xT[r0 : r0 + 128, :])
                    xt.append(t)
                for o2 in range(2):
                    c = k * 2 + o2
                    y_sb = ypool.tile([128, rows], out_dt, tag="yt")
                    for n4 in range(n_chunks):
                        ps = ppool.tile([128, NFREE], f32)
                        for i2 in range(2):
                            w0 = (k * 2 + i2) * 256 + o2 * 128
                            nc.tensor.matmul(
                                ps[:],
                                lhsT=w_all[:, w0 : w0 + 128],
                                rhs=xt[i2][:, n4 * NFREE : (n4 + 1) * NFREE],
                                start=(i2 == 0),
                                stop=(i2 == 1),
                            )
                        nc.scalar.activation(
                            y_sb[:, n4 * NFREE : (n4 + 1) * NFREE],
                            ps[:],
                            mybir.ActivationFunctionType.Identity,
                            bias=bias_sb[:, c : c + 1],
                        )
                    # y stores go out on the ACT HWDGE ring so they can't
                    # head-of-line-block the x loads on the SP ring.
                    nc.scalar.dma_start(
                        out=yT[c * 128 : (c + 1) * 128, :], in_=y_sb[:]
                    )

    nc.compile()
    return nc


def _get_nc(rows: int, mode: str):
    key = (rows, mode)
    if key not in _BUILT:
        _BUILT[key] = _build(rows, mode)
    return _BUILT[key]


def kernel(x: np.ndarray, W: np.ndarray, b: np.ndarray) -> np.ndarray:
    global LAST_EXEC_NS
    from concourse.bass_utils import run_bass_kernel_spmd

    assert x.shape == (BATCH, IN_FEATURES) and x.dtype == np.float32
    nc = _get_nc(ROWS, MODE)

    if MODE == "bf16":
        import ml_dtypes

        wire_dt = np.dtype(ml_dtypes.bfloat16)
    else:
        wire_dt = np.dtype(np.float32)

    # Weight image: Wh[p, (k*2+i2)*256 + o] = W[k, o, i2*128+p]
    Wh = np.ascontiguousarray(
        W.transpose(0, 2, 1)  # [k, i, o]
        .reshape(NUM_BLOCKS, 2, 128, BLOCK_OUT)  # [k, i2, p, o]
        .transpose(2, 0, 1, 3)  # [p, k, i2, o]
        .reshape(128, 32 * 256)
    ).astype(wire_dt)
    # Bias image: bh[p, k*2+o2] = b[k, o2*128+p]
    bh = np.ascontiguousarray(
        b.reshape(NUM_BLOCKS, 2, 128).transpose(2, 0, 1).reshape(128, 32)
    ).astype(np.float32, copy=False)

    xs = x.reshape(NCORES, ROWS, IN_FEATURES)
    in_maps = [
        {
            "xT": np.ascontiguousarray(xs[c].T).astype(wire_dt, copy=False),
            "Wh": Wh,
            "bh": bh,
        }
        for c in range(NCORES)
    ]

    res = run_bass_kernel_spmd(nc, in_maps, list(range(NCORES)), trace=TRACE)
    LAST_EXEC_NS = res.exec_time_ns

    y = np.empty((BATCH, OUT_FEATURES), dtype=np.float32)
    for c in range(NCORES):
        y[c * ROWS : (c + 1) * ROWS, :] = res.results[c]["yT"].T.astype(np.float32)
    return y


# revision 3
# speedup vs baseline: 2.0268x; 1.0368x over previous
"""Block-diagonal linear layer (16 blocks of 256x256) on 8 TRN2 NeuronCores.

Strategy: data-parallel over the batch dim (16384 rows -> 2048 rows/core).
The TensorEngine contracts over the partition dim, so x is pre-transposed on
the host to feature-major layout ([4096, rows]); the kernel computes
yT[o, n] = sum_i W[k, o, i] * xT[k*256+i, n] + b[k, o] per block and the host
transposes the gathered output back. Block weights are replicated, packed
host-side into a single [128, 8192] SBUF-ready image so they load in one DMA.

The kernel is memory-bound (71MB of f32 DMA traffic per core), so x/W/y are
carried as bf16 on the wire (PSUM accumulation stays f32); measured rel err
vs the f32 reference is ~2e-3.
"""

import sys

import numpy as np

try:
    import concourse  # noqa: F401
except ImportError:
    sys.path.insert(0, "/opt/trn_rl_repo")

NUM_BLOCKS = 16
IN_FEATURES = 4096
OUT_FEATURES = 4096
BLOCK_IN = 256
BLOCK_OUT = 256
BATCH = 16384
NCORES = 8
ROWS = BATCH // NCORES  # rows per core

# "bf16": x/W/y bf16 on the wire, f32 PSUM accumulate (fast, rel err ~2e-3)
# "f32r": everything f32, matmul in float32r mode (rel err ~1e-4)
MODE = "bf16"

# test.py toggles these for profiling.
TRACE = False
LAST_EXEC_NS = None

_BUILT = {}


def _build(rows: int, mode: str):
    """Build + compile the single-core Bass program (identical SPMD on 8 cores)."""
    import concourse.mybir as mybir
    import concourse.tile as tile
    from concourse import bacc

    nc = bacc.Bacc("TRN2", target_bir_lowering=False, debug=False)
    f32 = mybir.dt.float32
    mm_dt = mybir.dt.bfloat16 if mode == "bf16" else mybir.dt.float32r
    out_dt = mybir.dt.bfloat16 if mode == "bf16" else f32

    xT = nc.dram_tensor("xT", [IN_FEATURES, rows], mm_dt, kind="ExternalInput").ap()
    Wh = nc.dram_tensor("Wh", [128, 32 * 256], mm_dt, kind="ExternalInput").ap()
    bh = nc.dram_tensor("bh", [128, 32], f32, kind="ExternalInput").ap()
    yT = nc.dram_tensor("yT", [OUT_FEATURES, rows], out_dt, kind="ExternalOutput").ap()

    NFREE = 512  # one fp32 PSUM bank
    n_chunks = rows // NFREE

    with tile.TileContext(nc) as tc:
        with (
            tc.tile_pool(name="wp", bufs=1) as wpool,
            tc.tile_pool(name="xp", bufs=8) as xpool,
            tc.tile_pool(name="yp", bufs=6) as ypool,
            tc.tile_pool(name="pp", bufs=8, space="PSUM") as ppool,
        ):
            # Weights + bias ride the SWDGE (gpsimd) ring so the x loads on
            # the SP HWDGE ring start streaming at t=0 in parallel.
            w_all = wpool.tile([128, 32 * 256], mm_dt)
            nc.gpsimd.dma_start(out=w_all[:], in_=Wh[:])
            bias_sb = wpool.tile([128, 32], f32)
            nc.gpsimd.dma_start(out=bias_sb[:], in_=bh[:])

            for k in range(NUM_BLOCKS):
                xt = []
                for i2 in range(2):
                    t = xpool.tile([128, rows], mm_dt, tag="xt")
                    r0 = (k * 2 + i2) * 128
                    nc.sync.dma_start(out=t[:], in_=xT[r0 : r0 + 128, :])
                    xt.append(t)
                for o2 in range(2):
                    c = k * 2 + o2
                    y_sb = ypool.tile([128, rows], out_dt, tag="yt")
                    for n4 in range(n_chunks):
                        ps = ppool.tile([128, NFREE], f32)
                        for i2 in range(2):
                            w0 = (k * 2 + i2) * 256 + o2 * 128
                            nc.tensor.matmul(
                                ps[:],
                                lhsT=w_all[:, w0 : w0 + 128],
                                rhs=xt[i2][:, n4 * NFREE : (n4 + 1) * NFREE],
                                start=(i2 == 0),
                                stop=(i2 == 1),
                            )
                        # PSUM evacuation + bias add, split across ACT and
                        # DVE so neither engine becomes the wall.
                        y_slice = y_sb[:, n4 * NFREE : (n4 + 1) * NFREE]
                        if n4 % 2 == 0:
                            nc.scalar.activation(
                                y_slice,
                                ps[:],
                                mybir.ActivationFunctionType.Identity,
                                bias=bias_sb[:, c : c + 1],
                            )
                        else:
                            nc.vector.tensor_scalar_add(
                                y_slice, ps[:], bias_sb[:, c : c + 1]
                            )
                    # y stores alternate between the ACT HWDGE ring and the
                    # SWDGE ring; keeping them off the SP ring avoids
                    # head-of-line-blocking the x loads.
                    store_eng = nc.scalar if c % 2 == 0 else nc.gpsimd
                    store_eng.dma_start(
                        out=yT[c * 128 : (c + 1) * 128, :], in_=y_sb[:]
                    )

    nc.compile()
    return nc


def _get_nc(rows: int, mode: str):
    key = (rows, mode)
    if key not in _BUILT:
        _BUILT[key] = _build(rows, mode)
    return _BUILT[key]


def kernel(x: np.ndarray, W: np.ndarray, b: np.ndarray) -> np.ndarray:
    global LAST_EXEC_NS
    from concourse.bass_utils import run_bass_kernel_spmd

    assert x.shape == (BATCH, IN_FEATURES) and x.dtype == np.float32
    nc = _get_nc(ROWS, MODE)

    if MODE == "bf16":
        import ml_dtypes

        wire_dt = np.dtype(ml_dtypes.bfloat16)
    else:
        wire_dt = np.dtype(np.float32)

    # Weight image: Wh[p, (k*2+i2)*256 + o] = W[k, o, i2*128+p]
    Wh = np.ascontiguousarray(
        W.transpose(0, 2, 1)  # [k, i, o]
        .reshape(NUM_BLOCKS, 2, 128, BLOCK_OUT)  # [k, i2, p, o]
        .transpose(2, 0, 1, 3)  # [p, k, i2, o]
        .reshape(128, 32 * 256)
    ).astype(wire_dt)
    # Bias image: bh[p, k*2+o2] = b[k, o2*128+p]
    bh = np.ascontiguousarray(
        b.reshape(NUM_BLOCKS, 2, 128).transpose(2, 0, 1).reshape(128, 32)
    ).astype(np.float32, copy=False)

    xs = x.reshape(NCORES, ROWS, IN_FEATURES)
    in_maps = [
        {
            "xT": np.ascontiguousarray(xs[c].T).astype(wire_dt, copy=False),
            "Wh": Wh,
            "bh": bh,
        }
        for c in range(NCORES)
    ]

    res = run_bass_kernel_spmd(nc, in_maps, list(range(NCORES)), trace=TRACE)
    LAST_EXEC_NS = res.exec_time_ns

    y = np.empty((BATCH, OUT_FEATURES), dtype=np.float32)
    for c in range(NCORES):
        y[c * ROWS : (c + 1) * ROWS, :] = res.results[c]["yT"].T.astype(np.float32)
    return y
